# revision 2
# baseline (speedup 1.0000x reference)
"""EntailmentConeLoss on 8 Trainium2 NeuronCores — v3 (phase-separated).

Why this shape: HW experiments show (a) dma_gather is bound at ~3.4 ns per
gathered row (descriptor/bandwidth floor at 1KB f32 rows — smaller rows are
NOT faster), and (b) ANY DVE instruction running while SWDGE generates gather
descriptors causes a 3-12x super-additive stall (the Q7 descriptor generator
and the DVE share an exclusive SBUF port pair), while ACT overlaps gathers
cleanly (own ports). So:

- Rounds: buckets are packed into rounds of <= RC row-columns. Within a
  round, gathers stream in while ACT does fused Square+accumulate per column
  (pp and cc) — zero DVE activity. After the round's last gather, a DVE
  window computes the pc dot products (scalar_tensor_tensor mult+accumulate
  per column, or tensor_tensor + tensor_scalar-accum). A dummy DVE gate op
  that depends on the round's last gather keeps the whole DVE window out of
  the gather phase; round r+1's gathers reuse the row tiles (bufs=1) so they
  wait for the DVE window to drain.
- Table stays f32 (1KB rows are the gather sweet spot); bucket padding slots
  use trailing -1 indices with an exact num_idxs_reg so they cost nothing
  (first-use buckets pad with row 0 to keep SBUF contents finite).
- Epilogue (f32, both streams) runs in the final DVE window.
"""
import os
os.environ.setdefault("NEURON_RT_RESET_CORES", "1")

import numpy as np

C, D = 100000, 256
P_TOT, K = 65536, 4
NCORES = 8
PPC = P_TOT // NCORES
NPC = PPC * K
NBUCK = 16
EPS = np.float32(1e-6)
BETA = np.float32(0.1)
MARGIN = np.float32(0.1)
RC = 68                      # max row-columns held per round

_CACHE = {}


def _pack_rounds(spans_p, spans_n):
    """Pack (side, bucket) into rounds of <= RC columns; shared between the
    program builder and the host index prep (round-0 buckets pad with row 0
    instead of -1 because their row tiles start uninitialized)."""
    work = []
    colbase = 0
    for xy in range(NBUCK):
        work.append(("p", xy, spans_p[xy], colbase))
        colbase += spans_p[xy]
    colbase = 0
    for xy in range(NBUCK):
        work.append(("n", xy, spans_n[xy], colbase))
        colbase += spans_n[xy]
    rounds = []
    cur, cur_cols = [], 0
    for item in work:
        if cur_cols + item[2] > RC and cur:
            rounds.append(cur)
            cur, cur_cols = [], 0
        cur.append(item)
        cur_cols += item[2]
    if cur:
        rounds.append(cur)
    return rounds


def _build_program(spans_p, spans_n, cnts_p, cnts_n, loop_iters=1, stage=2,
                   dt_name="f32", pc_path="stt"):
    import concourse.bass as bass
    import concourse.bacc as bacc
    import concourse.mybir as mybir
    import concourse.tile as tile

    f16 = mybir.dt.float16
    f32 = mybir.dt.float32
    i16 = mybir.dt.int16
    Alu = mybir.AluOpType
    Act = mybir.ActivationFunctionType
    dt = f16 if dt_name == "f16" else f32

    NPCOL = sum(spans_p)
    NNCOL = sum(spans_n)

    nc = bacc.Bacc("TRN2", target_bir_lowering=False, num_devices=NCORES,
                   num_swdge_queues=4)
    table = nc.dram_tensor("tab", [C, D], dt, kind="ExternalInput")
    posa_i = nc.dram_tensor("posa_i", [128, NPCOL * 8], i16, kind="ExternalInput")
    posb_i = nc.dram_tensor("posb_i", [128, NPCOL * 8], i16, kind="ExternalInput")
    nega_i = nc.dram_tensor("nega_i", [128, NNCOL * 8], i16, kind="ExternalInput")
    negc_i = nc.dram_tensor("negc_i", [128, NNCOL * 8], i16, kind="ExternalInput")
    maskp = nc.dram_tensor("maskp", [128, NPCOL], f32, kind="ExternalInput")
    maskn = nc.dram_tensor("maskn", [128, NNCOL], f32, kind="ExternalInput")
    partials = nc.dram_tensor("partials", [128, 2], f32, kind="ExternalOutput")

    HALF_PI = float(np.float32(np.pi / 2))

    rounds0 = _pack_rounds(spans_p, spans_n)
    # attach counts; round-0 buckets gather their full capacity (row-0 pads)
    rounds = []
    for ri, rnd in enumerate(rounds0):
        out = []
        for side, xy, span, colbase in rnd:
            cnt = (cnts_p if side == "p" else cnts_n)[xy]
            if ri == 0:
                cnt = span * 128
            out.append((side, xy, span, cnt, colbase))
        rounds.append(out)

    with tile.TileContext(nc) as tc:
        with tc.tile_pool(name="io", bufs=1) as io, \
             tc.tile_pool(name="rows", bufs=1) as rowp, \
             tc.tile_pool(name="scr", bufs=2) as scrp, \
             tc.tile_pool(name="tmp", bufs=1) as tmp:

            posa_t = io.tile([128, NPCOL * 8], i16)
            posb_t = io.tile([128, NPCOL * 8], i16)
            nega_t = io.tile([128, NNCOL * 8], i16)
            negc_t = io.tile([128, NNCOL * 8], i16)
            maskp_t = io.tile([128, NPCOL], f32)
            maskn_t = io.tile([128, NNCOL], f32)
            nc.sync.dma_start(out=posa_t[:], in_=posa_i[:])
            nc.sync.dma_start(out=posb_t[:], in_=posb_i[:])
            nc.sync.dma_start(out=nega_t[:], in_=nega_i[:])
            nc.sync.dma_start(out=negc_t[:], in_=negc_i[:])
            nc.sync.dma_start(out=maskp_t[:], in_=maskp[:])
            nc.sync.dma_start(out=maskn_t[:], in_=maskn[:])

            pp_p = io.tile([128, NPCOL], f32)
            cc_p = io.tile([128, NPCOL], f32)
            pc_p = io.tile([128, NPCOL], f32)
            pp_n = io.tile([128, NNCOL], f32)
            cc_n = io.tile([128, NNCOL], f32)
            pc_n = io.tile([128, NNCOL], f32)
            out_t = io.tile([128, 2], f32)
            gate_t = io.tile([128, 1], f32)

            tview = table[:].rearrange("(q r) d -> q r d", r=4)

            qrr = [0]

            def loop_body(_i=None):
                for rnd in rounds:
                    rc = sum(it[2] for it in rnd)
                    a_rows = rowp.tile([128, RC, D], dt, tag="ra", name="ra")
                    c_rows = rowp.tile([128, RC, D], dt, tag="rc", name="rc")
                    roff = 0
                    gather_info = []   # (side, roff, w, colbase, cnt)
                    for side, xy, w, cnt, cb in rnd:
                        a_idx = posa_t if side == "p" else nega_t
                        c_idx = posb_t if side == "p" else negc_t
                        ca, cb4 = xy // 4, xy % 4
                        nc.gpsimd.dma_gather(
                            a_rows[:, roff:roff + w, :], tview[:, ca, :],
                            a_idx[:, cb * 8:(cb + w) * 8],
                            w * 128, cnt, D, elem_step=4 * D,
                            single_packet=False, queue_num=qrr[0] % 4)
                        nc.gpsimd.dma_gather(
                            c_rows[:, roff:roff + w, :], tview[:, cb4, :],
                            c_idx[:, cb * 8:(cb + w) * 8],
                            w * 128, cnt, D, elem_step=4 * D,
                            single_packet=False, queue_num=(qrr[0] + 1) % 4)
                        qrr[0] += 2
                        gather_info.append((side, roff, w, cb, cnt))
                        roff += w
                    rc_total = roff
                    if stage < 1:
                        continue
                    # ACT phase: fused Square+accum per column (pp and cc)
                    for side, roff, w, cb, cnt in gather_info:
                        pp_b = pp_p if side == "p" else pp_n
                        cc_b = cc_p if side == "p" else cc_n
                        for c in range(w):
                            ascr = scrp.tile([128, D], dt, tag="as", name="as")
                            nc.scalar.activation(
                                out=ascr[:], in_=a_rows[:, roff + c, :],
                                func=Act.Square,
                                accum_out=pp_b[:, cb + c:cb + c + 1])
                            cscr = scrp.tile([128, D], dt, tag="cs", name="cs")
                            nc.scalar.activation(
                                out=cscr[:], in_=c_rows[:, roff + c, :],
                                func=Act.Square,
                                accum_out=cc_b[:, cb + c:cb + c + 1])
                    # DVE gate: depends on the round's LAST gather, so every
                    # DVE op below (in-order on the DVE SEQ) stays out of the
                    # gather window.
                    nc.vector.tensor_scalar(
                        out=gate_t[:], in0=c_rows[:, rc_total - 1, 0:1],
                        scalar1=1.0, scalar2=None, op0=Alu.mult)
                    # DVE window: pc dot products
                    for side, roff, w, cb, cnt in gather_info:
                        pc_b = pc_p if side == "p" else pc_n
                        if pc_path == "stt":
                            for c in range(w):
                                sscr = scrp.tile([128, D], dt, tag="ss",
                                                 name="ss")
                                nc.vector.scalar_tensor_tensor(
                                    out=sscr[:], in0=a_rows[:, roff + c, :],
                                    scalar=1.0, in1=c_rows[:, roff + c, :],
                                    op0=Alu.mult, op1=Alu.mult,
                                    accum_out=pc_b[:, cb + c:cb + c + 1])
                        else:
                            prod = scrp.tile([128, w, D], dt, tag="pr",
                                             name="pr")
                            nc.vector.tensor_tensor(
                                out=prod[:, :w, :].rearrange("p a b -> p (a b)"),
                                in0=a_rows[:, roff:roff + w, :].rearrange("p a b -> p (a b)"),
                                in1=c_rows[:, roff:roff + w, :].rearrange("p a b -> p (a b)"),
                                op=Alu.mult)
                            for c in range(w):
                                nc.vector.tensor_scalar(
                                    out=prod[:, c, :], in0=prod[:, c, :],
                                    scalar1=1.0, scalar2=0.0,
                                    op0=Alu.mult, op1=Alu.add,
                                    accum_out=pc_b[:, cb + c:cb + c + 1])

                # ---------------- epilogue (wide f32 ops) ----------------
                def epilogue(pp_b, cc_b, pc_b, mask_t, ncol, is_neg, out_col):
                    T = lambda nm: tmp.tile([128, ncol], f32, tag="ep" + nm,
                                            name="ep" + nm)
                    ppcc = T("ppcc")
                    nc.vector.tensor_tensor(out=ppcc[:], in0=cc_b[:], in1=pp_b[:], op=Alu.add)
                    t2 = T("t2")
                    nc.vector.tensor_scalar(out=t2[:], in0=pc_b[:], scalar1=-2.0,
                                            scalar2=None, op0=Alu.mult)
                    dd = T("dd")
                    nc.vector.tensor_tensor(out=dd[:], in0=ppcc[:], in1=t2[:], op=Alu.add)
                    dupf = T("dupf")
                    thr = 1e-5 if dt_name == "f32" else 2e-3
                    nc.vector.tensor_scalar(out=dupf[:], in0=ppcc[:], scalar1=thr,
                                            scalar2=None, op0=Alu.mult)
                    nc.vector.tensor_tensor(out=dupf[:], in0=dd[:], in1=dupf[:], op=Alu.is_lt)
                    nc.vector.tensor_scalar(out=dupf[:], in0=dupf[:], scalar1=-1.0,
                                            scalar2=1.0, op0=Alu.mult, op1=Alu.add)
                    nc.vector.tensor_scalar(out=dd[:], in0=dd[:], scalar1=0.0,
                                            scalar2=None, op0=Alu.max)
                    g = T("g")
                    nc.vector.tensor_tensor(out=g[:], in0=pp_b[:], in1=dd[:], op=Alu.mult)
                    nc.vector.tensor_scalar(out=g[:], in0=g[:], scalar1=1e-30,
                                            scalar2=None, op0=Alu.add)
                    s0 = T("s0")
                    nc.scalar.activation(out=s0[:], in_=g[:], func=Act.Sqrt)
                    r = T("r")
                    nc.vector.reciprocal(r[:], s0[:])
                    s1 = T("s1")
                    nc.vector.tensor_tensor(out=s1[:], in0=g[:], in1=r[:], op=Alu.mult)
                    nc.vector.tensor_tensor(out=s1[:], in0=s1[:], in1=s0[:], op=Alu.add)
                    den = T("den")
                    nc.vector.tensor_scalar(out=den[:], in0=s1[:], scalar1=float(EPS),
                                            scalar2=None, op0=Alu.add)
                    rden = T("rden")
                    nc.vector.reciprocal(rden[:], den[:])
                    num = T("num")
                    nc.vector.tensor_tensor(out=num[:], in0=pc_b[:], in1=pp_b[:], op=Alu.subtract)
                    cos = T("cos")
                    nc.vector.tensor_tensor(out=cos[:], in0=num[:], in1=rden[:], op=Alu.mult)
                    nc.vector.tensor_scalar(out=cos[:], in0=cos[:], scalar1=2.0,
                                            scalar2=float(-(1.0 - 1e-6)), op0=Alu.mult,
                                            op1=Alu.max)
                    nc.vector.tensor_scalar(out=cos[:], in0=cos[:], scalar1=float(1.0 - 1e-6),
                                            scalar2=None, op0=Alu.min)
                    nc.vector.tensor_tensor(out=cos[:], in0=cos[:], in1=dupf[:], op=Alu.mult)
                    q = T("q")
                    nc.vector.tensor_tensor(out=q[:], in0=cos[:], in1=cos[:], op=Alu.mult)
                    nc.vector.tensor_scalar(out=q[:], in0=q[:], scalar1=-1.0,
                                            scalar2=1.0, op0=Alu.mult, op1=Alu.add)
                    q0 = T("q0")
                    nc.scalar.activation(out=q0[:], in_=q[:], func=Act.Sqrt)
                    rq = T("rq")
                    nc.vector.reciprocal(rq[:], q0[:])
                    sq = T("sq")
                    nc.vector.tensor_tensor(out=sq[:], in0=q[:], in1=rq[:], op=Alu.mult)
                    nc.vector.tensor_tensor(out=sq[:], in0=sq[:], in1=q0[:], op=Alu.add)
                    nc.vector.tensor_scalar(out=sq[:], in0=sq[:], scalar1=0.5,
                                            scalar2=None, op0=Alu.mult)
                    abst = T("abst")
                    nc.vector.tensor_scalar(out=abst[:], in0=cos[:], scalar1=-1.0,
                                            scalar2=None, op0=Alu.mult)
                    nc.vector.tensor_tensor(out=abst[:], in0=abst[:], in1=cos[:], op=Alu.max)
                    u = T("u")
                    nc.vector.tensor_tensor(out=u[:], in0=abst[:], in1=sq[:], op=Alu.min)
                    v = T("v")
                    nc.vector.tensor_tensor(out=v[:], in0=abst[:], in1=sq[:], op=Alu.max)
                    rv = T("rv")
                    nc.vector.reciprocal(rv[:], v[:])
                    rr = T("rr")
                    nc.vector.tensor_tensor(out=rr[:], in0=u[:], in1=rv[:], op=Alu.mult)
                    at = T("at")
                    nc.scalar.activation(out=at[:], in_=rr[:], func=Act.Arctan)
                    pg = T("pg")
                    nc.vector.tensor_scalar(out=pg[:], in0=cos[:], scalar1=0.0,
                                            scalar2=None, op0=Alu.is_gt)
                    ng = T("ng")
                    nc.vector.tensor_scalar(out=ng[:], in0=cos[:], scalar1=0.0,
                                            scalar2=None, op0=Alu.is_lt)
                    sgn = T("sgn")
                    nc.vector.tensor_tensor(out=sgn[:], in0=pg[:], in1=ng[:], op=Alu.subtract)
                    big = T("big")
                    nc.vector.tensor_tensor(out=big[:], in0=abst[:], in1=sq[:], op=Alu.is_gt)
                    c1 = T("c1")
                    nc.vector.tensor_scalar(out=c1[:], in0=big[:], scalar1=2.0,
                                            scalar2=-1.0, op0=Alu.mult, op1=Alu.add)
                    nc.vector.tensor_tensor(out=c1[:], in0=c1[:], in1=sgn[:], op=Alu.mult)
                    c0 = T("c0")
                    nc.vector.tensor_tensor(out=c0[:], in0=big[:], in1=ng[:], op=Alu.mult)
                    nc.vector.tensor_scalar(out=c0[:], in0=c0[:], scalar1=float(np.pi),
                                            scalar2=None, op0=Alu.mult)
                    c0b = T("c0b")
                    nc.vector.tensor_scalar(out=c0b[:], in0=big[:], scalar1=-HALF_PI,
                                            scalar2=HALF_PI, op0=Alu.mult, op1=Alu.add)
                    nc.vector.tensor_tensor(out=c0[:], in0=c0[:], in1=c0b[:], op=Alu.add)
                    ang = T("ang")
                    nc.vector.tensor_tensor(out=ang[:], in0=c1[:], in1=at[:], op=Alu.mult)
                    nc.vector.tensor_tensor(out=ang[:], in0=ang[:], in1=c0[:], op=Alu.add)
                    sp0 = T("sp0")
                    nc.scalar.activation(out=sp0[:], in_=pp_b[:], func=Act.Sqrt)
                    rp = T("rp")
                    nc.vector.reciprocal(rp[:], sp0[:])
                    sp1 = T("sp1")
                    nc.vector.tensor_tensor(out=sp1[:], in0=pp_b[:], in1=rp[:], op=Alu.mult)
                    nc.vector.tensor_tensor(out=sp1[:], in0=sp1[:], in1=sp0[:], op=Alu.add)
                    nc.vector.tensor_scalar(out=sp1[:], in0=sp1[:], scalar1=0.5,
                                            scalar2=float(EPS), op0=Alu.mult, op1=Alu.add)
                    rsp = T("rsp")
                    nc.vector.reciprocal(rsp[:], sp1[:])
                    y = T("y")
                    nc.vector.tensor_scalar(out=y[:], in0=rsp[:], scalar1=float(BETA),
                                            scalar2=0.0, op0=Alu.mult, op1=Alu.max)
                    nc.vector.tensor_scalar(out=y[:], in0=y[:], scalar1=float(1.0 - 1e-6),
                                            scalar2=None, op0=Alu.min)
                    y2 = T("y2")
                    nc.vector.tensor_tensor(out=y2[:], in0=y[:], in1=y[:], op=Alu.mult)
                    y3 = T("y3")
                    nc.vector.tensor_tensor(out=y3[:], in0=y2[:], in1=y[:], op=Alu.mult)
                    ap = T("ap")
                    nc.vector.tensor_scalar(out=ap[:], in0=y3[:], scalar1=float(1.0 / 6.0),
                                            scalar2=None, op0=Alu.mult)
                    nc.vector.tensor_tensor(out=ap[:], in0=ap[:], in1=y[:], op=Alu.add)
                    e = T("e")
                    nc.vector.tensor_tensor(out=e[:], in0=ang[:], in1=ap[:], op=Alu.subtract)
                    nc.vector.tensor_scalar(out=e[:], in0=e[:], scalar1=0.0,
                                            scalar2=None, op0=Alu.max)
                    if is_neg:
                        nc.vector.tensor_scalar(out=e[:], in0=e[:], scalar1=-1.0,
                                                scalar2=float(MARGIN), op0=Alu.mult,
                                                op1=Alu.add)
                        nc.vector.tensor_scalar(out=e[:], in0=e[:], scalar1=0.0,
                                                scalar2=None, op0=Alu.max)
                    nc.vector.tensor_tensor(out=e[:], in0=e[:], in1=mask_t[:], op=Alu.mult)
                    nc.vector.tensor_reduce(
                        out=out_t[:, out_col:out_col + 1], in_=e[:],
                        axis=mybir.AxisListType.X, op=Alu.add)

                if stage == 2:
                    epilogue(pp_p, cc_p, pc_p, maskp_t, NPCOL, False, 0)
                    epilogue(pp_n, cc_n, pc_n, maskn_t, NNCOL, True, 1)
                else:
                    nc.vector.tensor_reduce(out=out_t[:, 0:1], in_=maskp_t[:],
                                            axis=mybir.AxisListType.X, op=Alu.add)
                    nc.vector.tensor_reduce(out=out_t[:, 1:2], in_=maskn_t[:],
                                            axis=mybir.AxisListType.X, op=Alu.add)
                nc.sync.dma_start(out=partials[:], in_=out_t[:])

            if loop_iters > 1:
                with tc.For_i(0, loop_iters, 1):
                    loop_body()
            else:
                loop_body()

    nc.compile()
    return nc


def _wrap_idx(q):
    cap = q.shape[0]
    w = q.reshape(cap // 16, 16).T
    return np.tile(w, (8, 1))


def _prep_stream(a_vals, c_vals):
    """Bucket-sort by (a%4, c%4) with exact per-bucket col spans.

    Returns wrapped idx pair, mask, spans, counts. Padding slots up to the
    shared count use idx 0 (real row); the rest use -1 (skipped by
    num_idxs_reg / trailing-negative truncation) — the -1 tail is replaced
    with 0 for first-use buckets at pad time."""
    key = (a_vals % 4) * 4 + (c_vals % 4)
    order = np.argsort(key, kind="stable")
    counts = np.bincount(key, minlength=NBUCK)
    spans = [max(1, (int(c) + 127) // 128) for c in counts]
    a_parts, c_parts, m_parts = [], [], []
    off_src = 0
    for xy in range(NBUCK):
        cnt = int(counts[xy])
        cap = spans[xy] * 128
        seg = order[off_src:off_src + cnt]
        off_src += cnt
        a_q = np.full(cap, -1, np.int16)
        c_q = np.full(cap, -1, np.int16)
        m = np.zeros(cap, np.float32)
        a_q[:cnt] = (a_vals[seg] // 4).astype(np.int16)
        c_q[:cnt] = (c_vals[seg] // 4).astype(np.int16)
        m[:cnt] = 1.0
        a_parts.append(a_q)
        c_parts.append(c_q)
        m_parts.append(m)
    return a_parts, c_parts, m_parts, spans, [int(c) for c in counts]


def _prepare(prototypes, pairs, neg_c, dt_name="f32"):
    np_dt = np.float16 if dt_name == "f16" else np.float32
    tab = np.ascontiguousarray(
        np.asarray(prototypes, dtype=np.float32).astype(np_dt))
    pairs = np.asarray(pairs, dtype=np.int32)
    neg_c = np.asarray(neg_c, dtype=np.int32)

    per_core = []
    for k in range(NCORES):
        pk = pairs[k * PPC:(k + 1) * PPC]
        nk = neg_c[k * NPC:(k + 1) * NPC]
        a, b = pk[:, 0], pk[:, 1]
        na = np.repeat(a, K)
        per_core.append((_prep_stream(a, b), _prep_stream(na, nk)))

    # Shared program across cores: per-bucket max spans and max counts.
    spans_p = tuple(max(pc[0][3][i] for pc in per_core) for i in range(NBUCK))
    spans_n = tuple(max(pc[1][3][i] for pc in per_core) for i in range(NBUCK))
    cnts_p = tuple(max(pc[0][4][i] for pc in per_core) for i in range(NBUCK))
    cnts_n = tuple(max(pc[1][4][i] for pc in per_core) for i in range(NBUCK))

    round0 = {(side, xy) for side, xy, _, _ in _pack_rounds(spans_p, spans_n)[0]}

    def assemble(parts, spans_tgt, cnts_tgt, side):
        a_parts, c_parts, m_parts, spans, cnts = parts
        a_out, c_out, m_out = [], [], []
        for xy in range(NBUCK):
            cap = spans_tgt[xy] * 128
            a_q = np.full(cap, -1, np.int16)
            c_q = np.full(cap, -1, np.int16)
            m = np.zeros(cap, np.float32)
            n = a_parts[xy].shape[0]
            a_q[:n] = a_parts[xy]
            c_q[:n] = c_parts[xy]
            m[:n] = m_parts[xy]
            # pad up to the call's num_idxs_reg with row 0 gathers
            cc = cap if (side, xy) in round0 else cnts_tgt[xy]
            pad = (a_q[:cc] < 0)
            a_q[:cc][pad] = 0
            c_q[:cc][pad] = 0
            a_out.append(_wrap_idx(a_q))
            c_out.append(_wrap_idx(c_q))
            m_out.append(m.reshape(cap // 128, 128).T)
        return (np.ascontiguousarray(np.concatenate(a_out, axis=1)),
                np.ascontiguousarray(np.concatenate(c_out, axis=1)),
                np.ascontiguousarray(np.concatenate(m_out, axis=1)))

    in_maps = []
    for k in range(NCORES):
        pa, pb, mp = assemble(per_core[k][0], spans_p, cnts_p, "p")
        ng_a, ng_c, mn = assemble(per_core[k][1], spans_n, cnts_n, "n")
        in_maps.append({
            "tab": tab,
            "posa_i": pa, "posb_i": pb,
            "nega_i": ng_a, "negc_i": ng_c,
            "maskp": mp, "maskn": mn,
        })
    return spans_p, spans_n, cnts_p, cnts_n, in_maps


def kernel(prototypes, pairs, neg_c, dt_name="f32", pc_path="stt"):
    from concourse.bass_utils import run_bass_kernel_spmd

    spans_p, spans_n, cnts_p, cnts_n, in_maps = _prepare(
        prototypes, pairs, neg_c, dt_name)
    key = (spans_p, spans_n, cnts_p, cnts_n, dt_name, pc_path)
    if key not in _CACHE:
        _CACHE[key] = _build_program(spans_p, spans_n, cnts_p, cnts_n,
                                     dt_name=dt_name, pc_path=pc_path)
    nc = _CACHE[key]

    res = run_bass_kernel_spmd(nc, in_maps, core_ids=list(range(NCORES)))
    pos_sum = 0.0
    neg_sum = 0.0
    for k in range(NCORES):
        part = res.results[k]["partials"]
        pos_sum += float(part[:, 0].sum(dtype=np.float64))
        neg_sum += float(part[:, 1].sum(dtype=np.float64))
    loss = 0.5 * (pos_sum / P_TOT + neg_sum / (P_TOT * K))
    return np.float32(loss)
